# revision 1
# baseline (speedup 1.0000x reference)
"""Trainium2 Bass kernel for nn_GroupEncoder.

Computes, for full inputs
    x:  (32, 128, 128, 128) f32
    r:  (32, 128, 128, 32)  f32
    w1: (128, 32, 8, 16)    f32
    w2: (32, 16, 8, 16)     f32
the reference:
    y = einsum('nijx,nijr->nrx', x, r)
    u = relu(einsum('nrx,xrvh->nrvh', y, w1) / (128*128))
    out = einsum('ruvh,nrvh->nruv', w2, u)        # (32, 32, 16, 8)

Sharding: data-parallel over n across 8 NeuronCores (4 samples/core),
w1/w2 replicated.  Per core the i,j contraction is done as 128 fp32
matmuls per sample (K=i on partitions, x stationary, r moving),
accumulating y^T = [x, r] in PSUM.  The tiny head (w1 matmul + relu +
w2 matmul) runs per-r on the same core; w2 is expanded host-side to a
block-diagonal [vh, uv] matrix per r so the v-batched contraction is a
single matmul per r.
"""

import numpy as np

# Problem constants (hardcoded; kernel.py must be self-contained).
N, I, J = 32, 128, 128
XD, RD, UD, VD, HD = 128, 32, 16, 8, 16
NCORES = 8
NLOC = N // NCORES  # 4 samples per core
NORM = float(I * J)
JC = 64  # j-chunk per x DMA: [128, 64*128] f32 = 4 MiB per transfer

_cache = {}


def _build_nc():
    import concourse.mybir as mybir
    import concourse.tile as tile
    from concourse import bacc

    f32 = mybir.dt.float32
    Relu = mybir.ActivationFunctionType.Relu

    nc = bacc.Bacc(
        "TRN2", target_bir_lowering=False, debug=False, num_devices=NCORES
    )
    x_d = nc.dram_tensor("x", [NLOC, I, J * XD], f32, kind="ExternalInput").ap()
    r_d = nc.dram_tensor("r", [NLOC, I, J * RD], f32, kind="ExternalInput").ap()
    w1_d = nc.dram_tensor("w1", [XD, RD * VD * HD], f32, kind="ExternalInput").ap()
    w2_d = nc.dram_tensor(
        "w2bd", [VD * HD, RD * UD * VD], f32, kind="ExternalInput"
    ).ap()
    out_d = nc.dram_tensor(
        "out", [UD * VD, RD * NLOC], f32, kind="ExternalOutput"
    ).ap()

    # Two HWDGE rings (SP + ACT) so big-DMA completion bubbles on one ring
    # are covered by streaming on the other.
    rings = [nc.sync, nc.scalar]

    with tile.TileContext(nc) as tc:
        with (
            tc.tile_pool(name="xp", bufs=3) as xp,
            tc.tile_pool(name="rp", bufs=2) as rp,
            tc.tile_pool(name="wp", bufs=1) as wp,
            tc.tile_pool(name="pys", bufs=2, space="PSUM") as pys,
            tc.tile_pool(name="pep", bufs=1, space="PSUM") as pep,
        ):
            w1_sb = wp.tile([XD, RD * VD * HD], f32)
            nc.sync.dma_start(w1_sb[:, :], w1_d[:, :])
            w2_sb = wp.tile([VD * HD, RD * UD * VD], f32)
            nc.scalar.dma_start(w2_sb[:, :], w2_d[:, :])
            # y^T staging: [x, r, n]
            yT_sb = wp.tile([XD, RD, NLOC], f32)
            # u1 pre-relu accumulates across samples: [vh, (r n)]
            u1ps = pep.tile([VD * HD, RD * NLOC], f32)

            for n in range(NLOC):
                ypsum = pys.tile([XD, RD], f32)
                rt = rp.tile([I, J * RD], f32)
                rings[(n + 1) % 2].dma_start(rt[:, :], r_d[n, :, :])
                for c in range(J // JC):
                    xt = xp.tile([I, JC * XD], f32)
                    rings[c % 2].dma_start(
                        xt[:, :], x_d[n, :, c * JC * XD : (c + 1) * JC * XD]
                    )
                    for j in range(JC):
                        jj = c * JC + j
                        nc.tensor.matmul(
                            ypsum[:, :],
                            xt[:, j * XD : (j + 1) * XD],
                            rt[:, jj * RD : (jj + 1) * RD],
                            start=(jj == 0),
                            stop=(jj == J - 1),
                        )
                nc.scalar.copy(yT_sb[:, :, n], ypsum[:, :])
                # Stage 2 for this sample (overlaps next sample's DMA):
                # u1[vh, r*4+n] = sum_x w1[x, (r vh)] * y^T[x, r, n]
                for rr in range(RD):
                    nc.tensor.matmul(
                        u1ps[:, rr * NLOC + n : rr * NLOC + n + 1],
                        w1_sb[:, rr * VD * HD : (rr + 1) * VD * HD],
                        yT_sb[:, rr, n : n + 1],
                        start=True,
                        stop=True,
                    )

            u1_sb = wp.tile([VD * HD, RD * NLOC], f32)
            nc.scalar.activation(u1_sb[:, :], u1ps[:, :], Relu)
            u2ps = pep.tile([UD * VD, RD * NLOC], f32)
            for rr in range(RD):
                nc.tensor.matmul(
                    u2ps[:, rr * NLOC : (rr + 1) * NLOC],
                    w2_sb[:, rr * UD * VD : (rr + 1) * UD * VD],
                    u1_sb[:, rr * NLOC : (rr + 1) * NLOC],
                    start=True,
                    stop=True,
                )
            out_sb = wp.tile([UD * VD, RD * NLOC], f32)
            nc.scalar.copy(out_sb[:, :], u2ps[:, :])
            nc.sync.dma_start(out_d[:, :], out_sb[:, :])

    nc.compile()
    return nc


def _prep_in_maps(x, r, w1, w2):
    x = np.asarray(x, dtype=np.float32)
    r = np.asarray(r, dtype=np.float32)
    w1 = np.asarray(w1, dtype=np.float32)
    w2 = np.asarray(w2, dtype=np.float32)

    # Fold the 1/(i*j) normalization into w1.
    w1p = np.ascontiguousarray((w1 / NORM).reshape(XD, RD * VD * HD))
    # Block-diagonal expansion of w2 over v:
    # w2bd[(v h), r, (u v')] = w2[r, u, v, h] if v == v' else 0
    w2bd = np.zeros((RD, VD, HD, UD, VD), np.float32)
    for v in range(VD):
        w2bd[:, v, :, :, v] = np.transpose(w2[:, :, v, :], (0, 2, 1))
    w2bd = np.ascontiguousarray(
        w2bd.reshape(RD, VD * HD, UD * VD)
        .transpose(1, 0, 2)
        .reshape(VD * HD, RD * UD * VD)
    )

    in_maps = []
    for c in range(NCORES):
        in_maps.append(
            {
                "x": np.ascontiguousarray(
                    x[c * NLOC : (c + 1) * NLOC].reshape(NLOC, I, J * XD)
                ),
                "r": np.ascontiguousarray(
                    r[c * NLOC : (c + 1) * NLOC].reshape(NLOC, I, J * RD)
                ),
                "w1": w1p,
                "w2bd": w2bd,
            }
        )
    return in_maps


def _assemble(results):
    outs = []
    for c in range(NCORES):
        o = np.asarray(results[c]["out"], dtype=np.float32)  # [uv, (r n)]
        outs.append(o.reshape(UD, VD, RD, NLOC).transpose(3, 2, 0, 1))
    return np.ascontiguousarray(np.concatenate(outs, axis=0))


def run(x, r, w1, w2, **spmd_kwargs):
    """Build (cached), run on 8 cores, return (output, BassKernelResults)."""
    from concourse.bass_utils import run_bass_kernel_spmd

    if "nc" not in _cache:
        _cache["nc"] = _build_nc()
    nc = _cache["nc"]
    in_maps = _prep_in_maps(x, r, w1, w2)
    res = run_bass_kernel_spmd(
        nc, in_maps, core_ids=list(range(NCORES)), **spmd_kwargs
    )
    return _assemble(res.results), res


def kernel(x, r, w1, w2):
    out, _ = run(x, r, w1, w2)
    return out
